# revision 2
# baseline (speedup 1.0000x reference)
"""Data-parallel Trainium kernel for nn_Attention_CRF.

Full inputs in, full outputs out. The batch (2048) is sharded across the
8 NeuronCores (256 samples each); all parameters (<25 MB) are replicated.
The forward pass runs as a short pipeline of pmap-compiled stages; data
stays sharded on-device between stages.

Compiler-friendliness rewrites (neuronx-cc ICEs on gather/convolution):
  * embedding lookup  -> one-hot (VOCAB=30) matmul
  * SAME 1D conv(k=3) -> 3 shifted matmuls on a padded sequence
"""

import jax
import jax.numpy as jnp
import numpy as np
from jax import lax

B, L, VOCAB = 2048, 70, 30
N_CORES = 8

_stages = None


def _lstm_dir(x, p):
    xg = jnp.einsum('bld,gd->blg', x, p['Wih']) + p['b']  # [b, L, 4H]
    H = p['Whh'].shape[1]
    h0 = jnp.zeros((x.shape[0], H), x.dtype)

    def step(carry, g_t):
        h, c = carry
        g = g_t + h @ p['Whh'].T
        i, f, gg, o = jnp.split(g, 4, axis=-1)
        c = jax.nn.sigmoid(f) * c + jax.nn.sigmoid(i) * jnp.tanh(gg)
        h = jax.nn.sigmoid(o) * jnp.tanh(c)
        return (h, c), h

    _, hs = lax.scan(step, (h0, h0), jnp.swapaxes(xg, 0, 1))
    return jnp.swapaxes(hs, 0, 1)  # [b, L, H]


def _bilstm(x, pf, pb):
    hf = _lstm_dir(x, pf)
    hb = _lstm_dir(x[:, ::-1], pb)[:, ::-1]
    return jnp.concatenate([hf, hb], axis=-1)


def _conv_mm(x, w, b):
    # SAME 1D conv (k=3) via 3 shifted matmuls; x [b, L, C], w [3, C, C].
    xp = jnp.pad(x, ((0, 0), (1, 1), (0, 0)))
    y = xp[:, 0:L] @ w[0] + xp[:, 1:L + 1] @ w[1] + xp[:, 2:L + 2] @ w[2]
    return jax.nn.relu(y + b)


def _normed_linear(x, W):
    xn = x / jnp.maximum(jnp.linalg.norm(x, axis=1, keepdims=True), 1e-12)
    Wn = W / jnp.maximum(jnp.linalg.norm(W, axis=0, keepdims=True), 1e-12)
    return xn @ Wn


def _stage_embed(x, p):
    aux = x[:, 70:74]
    ids = x[:, :70]
    onehot = (ids[:, :, None] == jnp.arange(VOCAB, dtype=jnp.float32)[None, None, :])
    e = onehot.astype(jnp.float32) @ p['emb']             # [b, L, EMB]
    h = jax.nn.relu(e @ p['lin_w'] + p['lin_b'])          # [b, L, 124]
    return jnp.concatenate(
        [h, jnp.broadcast_to(aux[:, None, :], (x.shape[0], L, 4))], axis=2)


def _stage_lan(h, p):
    return _bilstm(h, p['lan_f'], p['lan_b'])             # [b, L, 128]


def _stage_attn_conv(h, hl, p):
    att = jax.nn.softmax(
        jnp.einsum('bld,bmd->blm', hl, hl) / jnp.sqrt(jnp.float32(hl.shape[-1])),
        axis=-1)
    h1 = jnp.einsum('blm,bmd->bld', att, hl)
    h2 = _conv_mm(_conv_mm(h, p['cnn0_w'], p['cnn0_b']), p['cnn1_w'], p['cnn1_b'])
    return h1 + h2


def _stage_lstm2(y, p):
    return _bilstm(y, p['lstm2_f'], p['lstm2_b'])         # [b, L, 512]


def _stage_heads(hh, x, p):
    msa_feat = x[:, 74:]
    u = jnp.tanh(hh @ p['fcn1_w'] + p['fcn1_b'])
    u = u @ p['fcn2_w'] + p['fcn2_b']
    cs = u @ p['fcn2b_w'] + p['fcn2b_b']
    a = jax.nn.softmax(hh, axis=1)
    pooled = jnp.einsum('bls,blt->bst', a, u).reshape(x.shape[0], -1)
    msa = _normed_linear(msa_feat @ p['msa_w'] + p['msa_b'], p['msa_nw'])
    o = jax.nn.relu(
        jnp.concatenate([pooled, msa], axis=1) @ p['fcn4_w'] + p['fcn4_b'])
    return _normed_linear(o, p['fcn5_nw']), cs


def _get_stages():
    global _stages
    if _stages is None:
        devs = jax.devices()[:N_CORES]
        pm = lambda f, n: jax.pmap(f, in_axes=(0,) * n + (None,), devices=devs)
        _stages = (pm(_stage_embed, 1), pm(_stage_lan, 1),
                   pm(_stage_attn_conv, 2), pm(_stage_lstm2, 1),
                   pm(_stage_heads, 2))
    return _stages


def kernel(input, params):
    embed, lan, attn_conv, lstm2, heads = _get_stages()
    x = np.asarray(input, dtype=np.float32).reshape(N_CORES, B // N_CORES, -1)
    h = embed(x, params)
    hl = lan(h, params)
    y = attn_conv(h, hl, params)
    hh = lstm2(y, params)
    out, cs = heads(hh, x, params)
    out = np.asarray(out).reshape(B, -1)
    cs = np.asarray(cs).reshape(B, L, -1)
    return out, cs
